# revision 34
# baseline (speedup 1.0000x reference)
"""Band-sparse (local block) attention on 8 TRN2 NeuronCores.

Problem: q,k,v [4096, 8, 64] f32; block size 128; banded block mask with 4
blocks each side of the diagonal (window 512). pair_bias is unused.

Sharding: one head per NeuronCore (8 heads / 8 cores).

Per-core algorithm (head h), panel-structured:
  Queries are processed in 8 panels of 512. Panel p touches key blocks
  c in [4p-4, 4p+7] (clipped); within the panel, block c covers the
  contiguous query subrange where |query_block - c| <= 4.

  QK (PE, 2x row tiling): kT for even key blocks lives in SBUF
  partitions 0-63, odd blocks in 64-127; qT is duplicated into both
  halves. Two K=64 matmuls (one even block, one odd block) execute
  concurrently on PE row-tiles (0,0)/(64,0), writing S^T chunks into
  different PSUM banks -> ~2x QK throughput despite D=64.

  S^T layout: per panel, full-width (512) blocks get one PSUM bank
  each; partial edge blocks are packed two-per-bank so every computed
  column is needed. Banks are grouped into [128, 1536] PSUM tiles.

  exp: full-block groups run on ScalarE (activation Exp, scale=1/8,
  1536-wide calls to amortize the ~352-cycle overhead). The partial
  group (exactly 3 of the 9 band blocks for every query) runs on
  VectorE as a Schraudolph exponential: int16(A*s + B) bitcast to
  bf16 approximates exp(s/8) with ~1.8% rms, quasi-random error that
  averages out in the softmax ratio. This splits the exp work (the
  single-engine bottleneck) across both engines.

  PV (PE, full 128x128): o_ps[65, 512] += vo_c^T @ P_c per block;
  row 64 accumulates the softmax denominator via a ones column in vo.
  Evacuate via DVE to SBUF f32, DMA to ot [65, 4096].

Host: out = (ot[:64] / ot[64:65]).T per head.
"""

import os
import sys

import numpy as np


def _ensure_path():
    try:
        import concourse  # noqa: F401
    except ImportError:
        for p in ("/opt/trn_rl_repo", "/root/.axon_site/_ro/trn_rl_repo"):
            if os.path.isdir(p) and p not in sys.path:
                sys.path.insert(0, p)


_ensure_path()

import ml_dtypes  # noqa: E402

import concourse.bacc as bacc  # noqa: E402
import concourse.tile as tile  # noqa: E402
from concourse import mybir  # noqa: E402
from concourse.bass_utils import run_bass_kernel_spmd  # noqa: E402

N, H, D, B = 4096, 8, 64, 128
NROW = N // B  # 32 key/row blocks
BPS = 4  # band: blocks per side
NPANEL = 8
PQ = N // NPANEL  # 512 queries per panel
SCALE = 1.0 / 8.0  # D ** -0.5
F32 = mybir.dt.float32
BF16 = mybir.dt.bfloat16
I16 = mybir.dt.int16
NP_BF16 = ml_dtypes.bfloat16

# Schraudolph bf16 exp: bits = round(A*s + B) as int16, viewed as bf16,
# approximates exp(s * SCALE). C chosen so the mean relative error is ~0
# (errors then average out in the softmax numerator/denominator).
SCH_C = 0.0525
SCH_A = 128.0 * SCALE * np.log2(np.e)
SCH_B = 128.0 * (127.0 - SCH_C)


def _panel_portions(p):
    """Block portions of panel p: list of (c, q_lo, q_hi), query offsets
    panel-relative. Full blocks have q_hi-q_lo == PQ."""
    out = []
    for c in range(max(0, 4 * p - BPS), min(NROW, 4 * p + 4 + BPS)):
        r_lo = max(4 * p, c - BPS)
        r_hi = min(4 * p + 3, c + BPS)
        if r_lo > r_hi:
            continue
        out.append((c, (r_lo - 4 * p) * B, (r_hi - 4 * p + 1) * B))
    return out


def _panel_layout(p):
    """PSUM bank layout for panel p.

    Returns (banks, groups):
      banks: list of bank descriptors, each a list of
             (c, q_lo, q_hi, col_off) entries (col_off = offset of the
             portion inside the bank).
      groups: list of (bank_lo, bank_hi, engine) - contiguous bank spans
              activated in one exp call; engine 'act' or 'dve'.
    """
    portions = _panel_portions(p)
    fulls = [x for x in portions if x[2] - x[1] == PQ]
    partials = [x for x in portions if x[2] - x[1] < PQ]
    # pack partials two-per-bank: pair widest prefix with narrowest
    # suffix (widths pair to <= 512 for every panel of this problem)
    partials = sorted(partials, key=lambda x: -(x[2] - x[1]))
    banks = []
    used = [False] * len(partials)
    for i, a in enumerate(partials):
        if used[i]:
            continue
        used[i] = True
        bank = [(a[0], a[1], a[2], 0)]
        w = a[2] - a[1]
        for j in range(len(partials) - 1, i, -1):
            bj = partials[j]
            if not used[j] and w + (bj[2] - bj[1]) <= PQ:
                used[j] = True
                bank.append((bj[0], bj[1], bj[2], w))
                w += bj[2] - bj[1]
                break
        banks.append(bank)
    pbanks = banks
    fbanks = [[(c, ql, qh, 0)] for (c, ql, qh) in fulls]
    if p < NPANEL - 1:
        banks = fbanks + pbanks
    else:
        banks = pbanks + fbanks  # last panel: partials are low-c prefix
    # groups: spans of <=3 banks; full banks on ACT, partial banks on DVE
    groups = []
    if p < NPANEL - 1:
        nf = len(fbanks)
        for lo in range(0, nf, 3):
            groups.append((lo, min(lo + 3, nf), "act"))
        groups.append((nf, len(banks), "dve"))
    else:
        npb = len(pbanks)
        groups.append((0, npb, "dve"))
        for lo in range(npb, len(banks), 3):
            groups.append((lo, min(lo + 3, len(banks)), "act"))
    return banks, groups


def _build_nc():
    nc = bacc.Bacc(None)
    qt_d = nc.dram_tensor("qt", [2 * D, N], BF16, kind="ExternalInput")
    kt_d = nc.dram_tensor("kt", [2 * D, NROW // 2, B], BF16, kind="ExternalInput")
    vo_d = nc.dram_tensor("vo", [B, NROW, D + 1], BF16, kind="ExternalInput")
    # fc: panel-0 first chunk = kt blocks 0-3 (cols 0-255) + qt cols
    # 0-511 (cols 256-767), both partition halves - ONE first DMA means
    # panel-0's QK data completes ~0.5us earlier (DMA completion has a
    # ~2us fixed receipt cost; fewer serialized issues win).
    fc_d = nc.dram_tensor("fc", [2 * D, 2 * B + 512], BF16, kind="ExternalInput")
    ot_d = nc.dram_tensor("ot", [D + 1, N], F32, kind="ExternalOutput")
    scr_d = nc.dram_tensor("scr", [B, 16], BF16, kind="Internal")

    with tile.TileContext(nc) as tc:
        with (
            tc.tile_pool(name="io", bufs=1) as io_pool,
            tc.tile_pool(name="pexp", bufs=6) as p_pool,
            tc.tile_pool(name="st", bufs=2, space="PSUM") as st_pool,
            tc.tile_pool(name="acc", bufs=2, space="PSUM") as acc_pool,
            tc.tile_pool(name="ev", bufs=2) as ev_pool,
        ):
            # HAM warmup: PE boots throttled at 1.2 GHz; ~3.4us of activity
            # un-throttles it. Burn dummy matmuls during the input-DMA wait.
            # Also preload the ACT exp table (~2.7us) off the critical path.
            wz = io_pool.tile([B, 512], BF16)
            nc.vector.memset(wz, 0.0)  # DVE: ready ~1.3us before gpsimd
            wexp = io_pool.tile([B, 16], BF16)
            nc.scalar.activation(
                wexp, wz[:, :16], mybir.ActivationFunctionType.Exp, scale=SCALE
            )
            # Warm the scalar HWDGE ring while ScalarE is idle: its first
            # DMA otherwise pays ~700ns of extra ring-setup at the very
            # end of the kernel (the final half-evac out-DMA).
            nc.scalar.dma_start(out=scr_d[:, :], in_=wexp)
            wps = st_pool.tile([B, 3 * 512], F32, name="st", tag="st")
            for _ in range(6):
                nc.tensor.matmul(
                    wps[:, :512], wz[:, :B], wz, start=True, stop=True
                )

            qt = io_pool.tile([2 * D, N], BF16)
            kt = io_pool.tile([2 * D, NROW // 2, B], BF16)
            vo = io_pool.tile([B, NROW, D + 1], BF16)
            fc = io_pool.tile([2 * D, 2 * B + 512], BF16)
            # Input DMAs: one queue, strictly in consumption order with
            # the combined panel-0 chunk first. A single queue completes
            # chunks in issue order; a second parallel queue's bulk
            # traffic would share the 16 SDMA engines and delay the
            # small panel-0 chunks (measured +4.5us).
            nc.sync.dma_start(out=fc, in_=fc_d[:, :])
            nc.sync.dma_start(out=kt[:, 2:4, :], in_=kt_d[:, 2:4, :])
            nc.sync.dma_start(out=vo[:, 0:8, :], in_=vo_d[:, 0:8, :])
            nc.sync.dma_start(out=qt[:, 512:1024], in_=qt_d[:, 512:1024])
            nc.sync.dma_start(out=kt[:, 4:16, :], in_=kt_d[:, 4:16, :])
            nc.sync.dma_start(out=qt[:, 1024:2560], in_=qt_d[:, 1024:2560])
            nc.sync.dma_start(out=vo[:, 8:32, :], in_=vo_d[:, 8:32, :])
            nc.sync.dma_start(out=qt[:, 2560:4096], in_=qt_d[:, 2560:4096])

            def kt_slice(c):
                par = c % 2
                if c < 4:
                    i = c // 2
                    return fc[par * D : (par + 1) * D, i * B : (i + 1) * B]
                return kt[par * D : (par + 1) * D, c // 2, :]

            def qt_slice(c, q_lo, q_hi):
                par = c % 2
                if q_hi <= 512:  # panel 0 queries live in the fc chunk
                    return fc[par * D : (par + 1) * D, 2 * B + q_lo : 2 * B + q_hi]
                return qt[par * D : (par + 1) * D, q_lo:q_hi]

            # Dummy-matmul target for HAM warm-keeping (write-only; shares
            # the o_ps tag so it borrows a PSUM bank before PV needs it).
            warm_ps = acc_pool.tile([B, PQ], F32, name="warm", tag="ops")

            # Per-panel state
            layouts = [_panel_layout(p) for p in range(NPANEL)]
            st_tiles = {}  # (p, gi) -> (psum tile, bank_lo)
            pc_tiles = {}  # (p, gi) -> sbuf tile
            o_ps = {}

            def qk_pairs(p):
                """(even,odd) QK matmul pair order for panel p; yields lists
                of (c, q_lo, q_hi, bank_idx, col_off).

                Portions are ordered by (group, c) so groups complete in
                allocation order (the st pool has 2 slots for 3 live
                groups - the 3rd group's QK must come after the 1st
                group's exp). Even/odd c run concurrently as PE row
                tiles; paired portions always hit different PSUM banks
                (asserted) so the concurrent writes are bank-disjoint."""
                banks, groups = layouts[p]
                gi_of_bank = {}
                for gi, (lo, hi, _e) in enumerate(groups):
                    for bi in range(lo, hi):
                        gi_of_bank[bi] = gi
                flat = []
                for bi, bank in enumerate(banks):
                    for (c, ql, qh, off) in bank:
                        flat.append((c, ql, qh, bi, off))
                flat.sort(key=lambda x: (gi_of_bank[x[3]], x[0]))
                evens = [x for x in flat if x[0] % 2 == 0]
                odds = [x for x in flat if x[0] % 2 == 1]
                pairs = []
                for i in range(max(len(evens), len(odds))):
                    pair = []
                    if i < len(evens):
                        pair.append(evens[i])
                    if i < len(odds):
                        pair.append(odds[i])
                    if len(pair) == 2:
                        assert pair[0][3] != pair[1][3], (p, pair)
                    pairs.append(pair)
                return pairs

            def emit_qk_pair(p, pair):
                banks, groups = layouts[p]
                for (c, ql, qh, bi, off) in pair:
                    # find the group tile containing bank bi
                    gi = next(
                        i for i, (lo, hi, _e) in enumerate(groups)
                        if lo <= bi < hi
                    )
                    if (p, gi) not in st_tiles:
                        lo, hi, _e = groups[gi]
                        stt = st_pool.tile(
                            [B, 3 * 512], F32, name="st", tag="st"
                        )
                        st_tiles[(p, gi)] = (stt, lo)
                    stt, bank_lo = st_tiles[(p, gi)]
                    col = (bi - bank_lo) * 512 + off
                    w = qh - ql
                    nc.tensor.matmul(
                        stt[:, col : col + w],
                        kt_slice(c),
                        qt_slice(c, p * PQ + ql, p * PQ + qh),
                        start=True,
                        stop=True,
                    )

            def emit_exp(p, gi):
                banks, groups = layouts[p]
                lo, hi, eng = groups[gi]
                stt, bank_lo = st_tiles[(p, gi)]
                # used width of this group (last bank may be partially full)
                last = banks[hi - 1]
                wlast = max(e[3] + (e[2] - e[1]) for e in last)
                w = (hi - 1 - lo) * 512 + wlast
                pc = p_pool.tile([B, 3 * 512], BF16, tag="pc")
                if eng == "act":
                    nc.scalar.activation(
                        pc[:, :w],
                        stt[:, :w],
                        mybir.ActivationFunctionType.Exp,
                        scale=SCALE,
                    )
                else:
                    nc.vector.tensor_scalar(
                        pc[:, :w].bitcast(I16),
                        stt[:, :w],
                        SCH_A,
                        SCH_B,
                        mybir.AluOpType.mult,
                        mybir.AluOpType.add,
                    )
                pc_tiles[(p, gi)] = pc

            def pv_items(p):
                banks, groups = layouts[p]
                items = []
                for gi, (lo, hi, _e) in enumerate(groups):
                    pc = pc_tiles[(p, gi)]
                    for bi in range(lo, hi):
                        for (c, ql, qh, off) in banks[bi]:
                            col = (bi - lo) * 512 + off
                            items.append((gi, c, ql, qh, pc, col))
                # group order = exp completion order (don't stall the
                # first PV matmul on the last group's exp)
                items.sort(key=lambda x: (x[0], x[1]))
                return [x[1:] for x in items]

            def emit_pv(p):
                items = pv_items(p)
                ops = acc_pool.tile([B, PQ], F32, name="ops", tag="ops")
                o_ps[p] = ops
                for i, (c, ql, qh, pc, col) in enumerate(items):
                    nc.tensor.matmul(
                        ops[: D + 1, ql:qh],
                        vo[:, c, :],
                        pc[:, col : col + (qh - ql)],
                        start=(i == 0),
                        stop=(i == len(items) - 1),
                    )

            def emit_evac(p):
                ev = ev_pool.tile([D + 1, PQ], F32, tag="ev")
                ops = o_ps[p]
                nc.vector.tensor_copy(ev, ops[: D + 1, :])
                nc.sync.dma_start(
                    out=ot_d[:, p * PQ : (p + 1) * PQ], in_=ev
                )

            def emit_pv_final(p):
                """Last panel: accumulate the two column halves into two
                separate 1-bank PSUM tiles so the first half's evac (ACT)
                can run while the PE is still accumulating the second
                half (different banks - no PSUM collision)."""
                items = pv_items(p)
                h = PQ // 2
                tiles = []
                for hlo, hhi in ((0, h), (h, PQ)):
                    chain = []
                    for (c, ql, qh, pc, col) in items:
                        a, b = max(ql, hlo), min(qh, hhi)
                        if a < b:
                            chain.append((c, a, b, pc, col + (a - ql)))
                    opsf = acc_pool.tile([B, h], F32, name="opsf", tag="ops")
                    tiles.append((hlo, opsf, chain))
                for (hlo, opsf, chain) in tiles:
                    for i, (c, a, b, pc, col) in enumerate(chain):
                        nc.tensor.matmul(
                            opsf[: D + 1, a - hlo : b - hlo],
                            vo[:, c, :],
                            pc[:, col : col + (b - a)],
                            start=(i == 0),
                            stop=(i == len(chain) - 1),
                        )
                return tiles

            def emit_evac_final(p, tiles):
                h = PQ // 2
                ev = ev_pool.tile([D + 1, PQ], F32, tag="ev")
                (_, t0, _), (_, t1, _) = tiles
                # half 0 is ready first (its PV chain finishes while the
                # PE still accumulates half 1) - put it on the idle sync
                # queue so its DMA starts immediately.
                nc.scalar.copy(ev[:, :h], t0[: D + 1, :])
                nc.sync.dma_start(
                    out=ot_d[:, p * PQ : p * PQ + h], in_=ev[:, :h]
                )
                nc.vector.tensor_copy(ev[:, h:], t1[: D + 1, :])
                nc.scalar.dma_start(
                    out=ot_d[:, p * PQ + h : (p + 1) * PQ], in_=ev[:, h:]
                )

            # Software pipeline: each group's exp is emitted right after
            # its banks are complete; PV(p-1) is injected into the middle
            # of panel p's QK stream so PE stays busy while ACT/DVE chew
            # panel p's exps, and ACT/DVE stay busy during PV.
            pair_lists = [qk_pairs(p) for p in range(NPANEL)]
            ngroups = [len(layouts[p][1]) for p in range(NPANEL)]

            for p in range(NPANEL):
                banks, groups = layouts[p]
                # which pair index completes each group's banks
                done_after = []
                for gi, (lo, hi, _e) in enumerate(groups):
                    group_cs = {
                        e[0] for bi in range(lo, hi) for e in banks[bi]
                    }
                    last_pair = max(
                        pi
                        for pi, pair in enumerate(pair_lists[p])
                        if any(x[0] in group_cs for x in pair)
                    )
                    done_after.append(last_pair)
                inject_at = min(2, len(pair_lists[p]) - 1)
                for pi, pair in enumerate(pair_lists[p]):
                    emit_qk_pair(p, pair)
                    for gi in range(ngroups[p]):
                        if done_after[gi] == pi:
                            emit_exp(p, gi)
                    if p > 0 and pi == inject_at:
                        emit_pv(p - 1)
                        emit_evac(p - 1)
                    # Panel 0/1 startup: the pipeline hasn't filled, so
                    # the PE has real dependency bubbles (3rd PSUM group
                    # waits on the 1st group's exp). Fill them with dummy
                    # matmuls so the HAM activity window stays busy and
                    # the PE un-throttles to 2.4 GHz ~7us sooner.
                    # Placement matters: the PE queue is FIFO, so dummies
                    # only fill a wait if emitted BEFORE the blocked
                    # instruction - (0,1) covers pair2's wait on exp-A(p0)
                    # (~1.7us), (0,3) covers panel-1's wait on exp-B(p0).
                    nwarm = {(0, 1): 4, (0, 3): 2, (1, 1): 1, (1, 3): 1,
                             (2, 1): 1}
                    for _ in range(nwarm.get((p, pi), 0)):
                        nc.tensor.matmul(
                            warm_ps, wz[:, :B], wz, start=True, stop=True
                        )
            final_tiles = emit_pv_final(NPANEL - 1)
            emit_evac_final(NPANEL - 1, final_tiles)

    nc.compile()
    return nc


_NC = None


def _get_nc():
    global _NC
    if _NC is None:
        _NC = _build_nc()
    return _NC


def _make_in_maps(q, k, v):
    q = np.ascontiguousarray(q, dtype=np.float32)
    k = np.ascontiguousarray(k, dtype=np.float32)
    v = np.ascontiguousarray(v, dtype=np.float32)
    in_maps = []
    for h in range(H):
        qT = q[:, h, :].T.astype(NP_BF16)  # [64, 4096]
        qt2 = np.ascontiguousarray(np.concatenate([qT, qT], axis=0))
        kT = k[:, h, :].T.astype(NP_BF16)  # [64, 4096]
        kb = kT.reshape(D, NROW, B)
        kt2 = np.ascontiguousarray(
            np.concatenate([kb[:, 0::2, :], kb[:, 1::2, :]], axis=0)
        )  # [128, 16, 128]: rows 0-63 even blocks, 64-127 odd blocks
        vb = v[:, h, :].reshape(NROW, B, D).transpose(1, 0, 2)  # [128, 32, 64]
        vo = np.concatenate(
            [vb, np.ones((B, NROW, 1), np.float32)], axis=2
        ).astype(NP_BF16)  # [128, 32, 65]
        fc = np.ascontiguousarray(
            np.concatenate(
                [kt2.reshape(2 * D, (NROW // 2) * B)[:, : 2 * B], qt2[:, :512]],
                axis=1,
            )
        )  # [128, 768]: kt blocks 0-3 | qt cols 0-511
        in_maps.append(
            {"qt": qt2, "kt": kt2, "vo": np.ascontiguousarray(vo), "fc": fc}
        )
    return in_maps


def run(q, k, v, trace=False, **trace_kwargs):
    """Returns (out [4096, 8, 64] f32, BassKernelResults)."""
    nc = _get_nc()
    in_maps = _make_in_maps(q, k, v)
    res = run_bass_kernel_spmd(
        nc, in_maps, list(range(H)), trace=trace, **trace_kwargs
    )
    out = np.empty((N, H, D), dtype=np.float32)
    for h in range(H):
        ot = res.results[h]["ot"]  # [65, 4096]
        out[:, h, :] = (ot[:D] / ot[D : D + 1]).T
    return out, res


def kernel(q, k, v, pair_bias=None):
    out, _ = run(q, k, v)
    return out


# revision 36
# speedup vs baseline: 1.0375x; 1.0375x over previous
"""Band-sparse (local block) attention on 8 TRN2 NeuronCores.

Problem: q,k,v [4096, 8, 64] f32; block size 128; banded block mask with 4
blocks each side of the diagonal (window 512). pair_bias is unused.

Sharding: one head per NeuronCore (8 heads / 8 cores).

Per-core algorithm (head h), panel-structured:
  Queries are processed in 8 panels of 512. Panel p touches key blocks
  c in [4p-4, 4p+7] (clipped); within the panel, block c covers the
  contiguous query subrange where |query_block - c| <= 4.

  QK (PE, 2x row tiling): kT for even key blocks lives in SBUF
  partitions 0-63, odd blocks in 64-127; qT is duplicated into both
  halves. Two K=64 matmuls (one even block, one odd block) execute
  concurrently on PE row-tiles (0,0)/(64,0), writing S^T chunks into
  different PSUM banks -> ~2x QK throughput despite D=64.

  S^T layout: per panel, full-width (512) blocks get one PSUM bank
  each; partial edge blocks are packed two-per-bank so every computed
  column is needed. Banks are grouped into [128, 1536] PSUM tiles.

  exp: full-block groups run on ScalarE (activation Exp, scale=1/8,
  1536-wide calls to amortize the ~352-cycle overhead). The partial
  group (exactly 3 of the 9 band blocks for every query) runs on
  VectorE as a Schraudolph exponential: int16(A*s + B) bitcast to
  bf16 approximates exp(s/8) with ~1.8% rms, quasi-random error that
  averages out in the softmax ratio. This splits the exp work (the
  single-engine bottleneck) across both engines.

  PV (PE, full 128x128): o_ps[65, 512] += vo_c^T @ P_c per block;
  row 64 accumulates the softmax denominator via a ones column in vo.
  Evacuate via DVE to SBUF f32, DMA to ot [65, 4096].

Host: out = (ot[:64] / ot[64:65]).T per head.
"""

import os
import sys

import numpy as np


def _ensure_path():
    try:
        import concourse  # noqa: F401
    except ImportError:
        for p in ("/opt/trn_rl_repo", "/root/.axon_site/_ro/trn_rl_repo"):
            if os.path.isdir(p) and p not in sys.path:
                sys.path.insert(0, p)


_ensure_path()

import ml_dtypes  # noqa: E402

import concourse.bacc as bacc  # noqa: E402
import concourse.tile as tile  # noqa: E402
from concourse import mybir  # noqa: E402
from concourse.bass_utils import run_bass_kernel_spmd  # noqa: E402

N, H, D, B = 4096, 8, 64, 128
NROW = N // B  # 32 key/row blocks
BPS = 4  # band: blocks per side
NPANEL = 8
PQ = N // NPANEL  # 512 queries per panel
SCALE = 1.0 / 8.0  # D ** -0.5
F32 = mybir.dt.float32
BF16 = mybir.dt.bfloat16
I16 = mybir.dt.int16
NP_BF16 = ml_dtypes.bfloat16

# Schraudolph bf16 exp: bits = round(A*s + B) as int16, viewed as bf16,
# approximates exp(s * SCALE). C chosen so the mean relative error is ~0
# (errors then average out in the softmax numerator/denominator).
SCH_C = 0.0525
SCH_A = 128.0 * SCALE * np.log2(np.e)
SCH_B = 128.0 * (127.0 - SCH_C)


def _panel_portions(p):
    """Block portions of panel p: list of (c, q_lo, q_hi), query offsets
    panel-relative. Full blocks have q_hi-q_lo == PQ."""
    out = []
    for c in range(max(0, 4 * p - BPS), min(NROW, 4 * p + 4 + BPS)):
        r_lo = max(4 * p, c - BPS)
        r_hi = min(4 * p + 3, c + BPS)
        if r_lo > r_hi:
            continue
        out.append((c, (r_lo - 4 * p) * B, (r_hi - 4 * p + 1) * B))
    return out


def _panel_layout(p):
    """PSUM bank layout for panel p.

    Returns (banks, groups):
      banks: list of bank descriptors, each a list of
             (c, q_lo, q_hi, col_off) entries (col_off = offset of the
             portion inside the bank).
      groups: list of (bank_lo, bank_hi, engine) - contiguous bank spans
              activated in one exp call; engine 'act' or 'dve'.
    """
    portions = _panel_portions(p)
    fulls = [x for x in portions if x[2] - x[1] == PQ]
    partials = [x for x in portions if x[2] - x[1] < PQ]
    # pack partials two-per-bank: pair widest prefix with narrowest
    # suffix (widths pair to <= 512 for every panel of this problem)
    partials = sorted(partials, key=lambda x: -(x[2] - x[1]))
    banks = []
    used = [False] * len(partials)
    for i, a in enumerate(partials):
        if used[i]:
            continue
        used[i] = True
        bank = [(a[0], a[1], a[2], 0)]
        w = a[2] - a[1]
        for j in range(len(partials) - 1, i, -1):
            bj = partials[j]
            if not used[j] and w + (bj[2] - bj[1]) <= PQ:
                used[j] = True
                bank.append((bj[0], bj[1], bj[2], w))
                w += bj[2] - bj[1]
                break
        banks.append(bank)
    pbanks = banks
    fbanks = [[(c, ql, qh, 0)] for (c, ql, qh) in fulls]
    if p < NPANEL - 1:
        banks = fbanks + pbanks
    else:
        banks = pbanks + fbanks  # last panel: partials are low-c prefix
    # groups: spans of <=3 banks; full banks on ACT, partial banks on DVE
    groups = []
    if p < NPANEL - 1:
        nf = len(fbanks)
        for lo in range(0, nf, 3):
            groups.append((lo, min(lo + 3, nf), "act"))
        groups.append((nf, len(banks), "dve"))
    else:
        npb = len(pbanks)
        groups.append((0, npb, "dve"))
        for lo in range(npb, len(banks), 3):
            groups.append((lo, min(lo + 3, len(banks)), "act"))
    return banks, groups


def _build_nc():
    nc = bacc.Bacc(None)
    qt_d = nc.dram_tensor("qt", [2 * D, N], BF16, kind="ExternalInput")
    kt_d = nc.dram_tensor("kt", [2 * D, NROW // 2, B], BF16, kind="ExternalInput")
    vo_d = nc.dram_tensor("vo", [B, NROW, D + 1], BF16, kind="ExternalInput")
    # fc: panel-0 first chunk = kt blocks 0-3 (cols 0-255) + qt cols
    # 0-511 (cols 256-767), both partition halves - ONE first DMA means
    # panel-0's QK data completes ~0.5us earlier (DMA completion has a
    # ~2us fixed receipt cost; fewer serialized issues win).
    fc_d = nc.dram_tensor("fc", [2 * D, 2 * B + 512], BF16, kind="ExternalInput")
    ot_d = nc.dram_tensor("ot", [D + 1, N], F32, kind="ExternalOutput")
    scr_d = nc.dram_tensor("scr", [B, 16], BF16, kind="Internal")

    with tile.TileContext(nc) as tc:
        with (
            tc.tile_pool(name="io", bufs=1) as io_pool,
            tc.tile_pool(name="pexp", bufs=6) as p_pool,
            tc.tile_pool(name="st", bufs=2, space="PSUM") as st_pool,
            tc.tile_pool(name="acc", bufs=2, space="PSUM") as acc_pool,
            tc.tile_pool(name="ev", bufs=2) as ev_pool,
        ):
            # HAM warmup: PE boots throttled at 1.2 GHz; ~3.4us of activity
            # un-throttles it. Burn dummy matmuls during the input-DMA wait.
            # Also preload the ACT exp table (~2.7us) off the critical path.
            wz = io_pool.tile([B, 512], BF16)
            nc.vector.memset(wz, 0.0)  # DVE: ready ~1.3us before gpsimd
            wexp = io_pool.tile([B, 16], BF16)
            nc.scalar.activation(
                wexp, wz[:, :16], mybir.ActivationFunctionType.Exp, scale=SCALE
            )
            # Warm the scalar HWDGE ring while ScalarE is idle: its first
            # DMA otherwise pays ~700ns of extra ring-setup at the very
            # end of the kernel (the final half-evac out-DMA).
            nc.scalar.dma_start(out=scr_d[:, :], in_=wexp)
            wps = st_pool.tile([B, 3 * 512], F32, name="st", tag="st")
            for _ in range(6):
                nc.tensor.matmul(
                    wps[:, :512], wz[:, :B], wz, start=True, stop=True
                )

            qt = io_pool.tile([2 * D, N], BF16)
            kt = io_pool.tile([2 * D, NROW // 2, B], BF16)
            vo = io_pool.tile([B, NROW, D + 1], BF16)
            fc = io_pool.tile([2 * D, 2 * B + 512], BF16)
            # Input DMAs: one queue, strictly in consumption order with
            # the combined panel-0 chunk first. A single queue completes
            # chunks in issue order; a second parallel queue's bulk
            # traffic would share the 16 SDMA engines and delay the
            # small panel-0 chunks (measured +4.5us).
            nc.sync.dma_start(out=fc, in_=fc_d[:, :])
            nc.sync.dma_start(out=kt[:, 2:4, :], in_=kt_d[:, 2:4, :])
            nc.sync.dma_start(out=vo[:, 0:8, :], in_=vo_d[:, 0:8, :])
            nc.sync.dma_start(out=qt[:, 512:1024], in_=qt_d[:, 512:1024])
            nc.sync.dma_start(out=kt[:, 4:16, :], in_=kt_d[:, 4:16, :])
            nc.sync.dma_start(out=qt[:, 1024:2560], in_=qt_d[:, 1024:2560])
            nc.sync.dma_start(out=vo[:, 8:32, :], in_=vo_d[:, 8:32, :])
            nc.sync.dma_start(out=qt[:, 2560:4096], in_=qt_d[:, 2560:4096])

            def kt_slice(c):
                par = c % 2
                if c < 4:
                    i = c // 2
                    return fc[par * D : (par + 1) * D, i * B : (i + 1) * B]
                return kt[par * D : (par + 1) * D, c // 2, :]

            def qt_slice(c, q_lo, q_hi):
                par = c % 2
                if q_hi <= 512:  # panel 0 queries live in the fc chunk
                    return fc[par * D : (par + 1) * D, 2 * B + q_lo : 2 * B + q_hi]
                return qt[par * D : (par + 1) * D, q_lo:q_hi]

            # Dummy-matmul target for HAM warm-keeping (write-only; shares
            # the o_ps tag so it borrows a PSUM bank before PV needs it).
            warm_ps = acc_pool.tile([B, PQ], F32, name="warm", tag="ops")

            # Per-panel state
            layouts = [_panel_layout(p) for p in range(NPANEL)]
            st_tiles = {}  # (p, gi) -> (psum tile, bank_lo)
            pc_tiles = {}  # (p, gi) -> sbuf tile
            o_ps = {}

            def qk_pairs(p):
                """(even,odd) QK matmul pair order for panel p; yields lists
                of (c, q_lo, q_hi, bank_idx, col_off).

                Portions are ordered by (group, c) so groups complete in
                allocation order (the st pool has 2 slots for 3 live
                groups - the 3rd group's QK must come after the 1st
                group's exp). Even/odd c run concurrently as PE row
                tiles; paired portions always hit different PSUM banks
                (asserted) so the concurrent writes are bank-disjoint."""
                banks, groups = layouts[p]
                gi_of_bank = {}
                for gi, (lo, hi, _e) in enumerate(groups):
                    for bi in range(lo, hi):
                        gi_of_bank[bi] = gi
                flat = []
                for bi, bank in enumerate(banks):
                    for (c, ql, qh, off) in bank:
                        flat.append((c, ql, qh, bi, off))
                flat.sort(key=lambda x: (gi_of_bank[x[3]], x[0]))
                evens = [x for x in flat if x[0] % 2 == 0]
                odds = [x for x in flat if x[0] % 2 == 1]
                pairs = []
                for i in range(max(len(evens), len(odds))):
                    pair = []
                    if i < len(evens):
                        pair.append(evens[i])
                    if i < len(odds):
                        pair.append(odds[i])
                    if len(pair) == 2:
                        assert pair[0][3] != pair[1][3], (p, pair)
                    pairs.append(pair)
                return pairs

            def emit_qk_pair(p, pair):
                banks, groups = layouts[p]
                for (c, ql, qh, bi, off) in pair:
                    # find the group tile containing bank bi
                    gi = next(
                        i for i, (lo, hi, _e) in enumerate(groups)
                        if lo <= bi < hi
                    )
                    if (p, gi) not in st_tiles:
                        lo, hi, _e = groups[gi]
                        stt = st_pool.tile(
                            [B, 3 * 512], F32, name="st", tag="st"
                        )
                        st_tiles[(p, gi)] = (stt, lo)
                    stt, bank_lo = st_tiles[(p, gi)]
                    col = (bi - bank_lo) * 512 + off
                    w = qh - ql
                    nc.tensor.matmul(
                        stt[:, col : col + w],
                        kt_slice(c),
                        qt_slice(c, p * PQ + ql, p * PQ + qh),
                        start=True,
                        stop=True,
                    )

            def emit_exp(p, gi):
                banks, groups = layouts[p]
                lo, hi, eng = groups[gi]
                stt, bank_lo = st_tiles[(p, gi)]
                # used width of this group (last bank may be partially full)
                last = banks[hi - 1]
                wlast = max(e[3] + (e[2] - e[1]) for e in last)
                w = (hi - 1 - lo) * 512 + wlast
                pc = p_pool.tile([B, 3 * 512], BF16, tag="pc")
                if eng == "act":
                    nc.scalar.activation(
                        pc[:, :w],
                        stt[:, :w],
                        mybir.ActivationFunctionType.Exp,
                        scale=SCALE,
                    )
                else:
                    nc.vector.tensor_scalar(
                        pc[:, :w].bitcast(I16),
                        stt[:, :w],
                        SCH_A,
                        SCH_B,
                        mybir.AluOpType.mult,
                        mybir.AluOpType.add,
                    )
                pc_tiles[(p, gi)] = pc

            def pv_items(p):
                banks, groups = layouts[p]
                items = []
                for gi, (lo, hi, _e) in enumerate(groups):
                    pc = pc_tiles[(p, gi)]
                    for bi in range(lo, hi):
                        for (c, ql, qh, off) in banks[bi]:
                            col = (bi - lo) * 512 + off
                            items.append((gi, c, ql, qh, pc, col))
                # group order = exp completion order (don't stall the
                # first PV matmul on the last group's exp)
                items.sort(key=lambda x: (x[0], x[1]))
                return [x[1:] for x in items]

            def emit_pv(p):
                items = pv_items(p)
                ops = acc_pool.tile([B, PQ], F32, name="ops", tag="ops")
                o_ps[p] = ops
                for i, (c, ql, qh, pc, col) in enumerate(items):
                    nc.tensor.matmul(
                        ops[: D + 1, ql:qh],
                        vo[:, c, :],
                        pc[:, col : col + (qh - ql)],
                        start=(i == 0),
                        stop=(i == len(items) - 1),
                    )

            def emit_evac(p):
                ev = ev_pool.tile([D + 1, PQ], F32, tag="ev")
                ops = o_ps[p]
                nc.vector.tensor_copy(ev, ops[: D + 1, :])
                nc.sync.dma_start(
                    out=ot_d[:, p * PQ : (p + 1) * PQ], in_=ev
                )

            def emit_pv_final(p):
                """Last panel: accumulate the two column halves into two
                separate 1-bank PSUM tiles so the first half's evac (ACT)
                can run while the PE is still accumulating the second
                half (different banks - no PSUM collision)."""
                items = pv_items(p)
                h = PQ // 2
                tiles = []
                for hlo, hhi in ((0, h), (h, PQ)):
                    chain = []
                    for (c, ql, qh, pc, col) in items:
                        a, b = max(ql, hlo), min(qh, hhi)
                        if a < b:
                            chain.append((c, a, b, pc, col + (a - ql)))
                    opsf = acc_pool.tile([B, h], F32, name="opsf", tag="ops")
                    tiles.append((hlo, opsf, chain))
                for (hlo, opsf, chain) in tiles:
                    for i, (c, a, b, pc, col) in enumerate(chain):
                        nc.tensor.matmul(
                            opsf[: D + 1, a - hlo : b - hlo],
                            vo[:, c, :],
                            pc[:, col : col + (b - a)],
                            start=(i == 0),
                            stop=(i == len(chain) - 1),
                        )
                return tiles

            def emit_evac_final(p, tiles):
                h = PQ // 2
                ev = ev_pool.tile([D + 1, PQ], F32, tag="ev")
                (_, t0, _), (_, t1, _) = tiles
                nc.scalar.copy(ev[:, :h], t0[: D + 1, :])
                nc.scalar.dma_start(
                    out=ot_d[:, p * PQ : p * PQ + h], in_=ev[:, :h]
                )
                nc.vector.tensor_copy(ev[:, h:], t1[: D + 1, :])
                nc.sync.dma_start(
                    out=ot_d[:, p * PQ + h : (p + 1) * PQ], in_=ev[:, h:]
                )

            # Software pipeline: each group's exp is emitted right after
            # its banks are complete; PV(p-1) is injected into the middle
            # of panel p's QK stream so PE stays busy while ACT/DVE chew
            # panel p's exps, and ACT/DVE stay busy during PV.
            pair_lists = [qk_pairs(p) for p in range(NPANEL)]
            ngroups = [len(layouts[p][1]) for p in range(NPANEL)]

            for p in range(NPANEL):
                banks, groups = layouts[p]
                # which pair index completes each group's banks
                done_after = []
                for gi, (lo, hi, _e) in enumerate(groups):
                    group_cs = {
                        e[0] for bi in range(lo, hi) for e in banks[bi]
                    }
                    last_pair = max(
                        pi
                        for pi, pair in enumerate(pair_lists[p])
                        if any(x[0] in group_cs for x in pair)
                    )
                    done_after.append(last_pair)
                inject_at = min(2, len(pair_lists[p]) - 1)
                for pi, pair in enumerate(pair_lists[p]):
                    emit_qk_pair(p, pair)
                    for gi in range(ngroups[p]):
                        if done_after[gi] == pi:
                            emit_exp(p, gi)
                    if p > 0 and pi == inject_at:
                        emit_pv(p - 1)
                        emit_evac(p - 1)
                    # Panel 0/1 startup: the pipeline hasn't filled, so
                    # the PE has real dependency bubbles (3rd PSUM group
                    # waits on the 1st group's exp). Fill them with dummy
                    # matmuls so the HAM activity window stays busy and
                    # the PE un-throttles to 2.4 GHz ~7us sooner.
                    # Placement matters: the PE queue is FIFO, so dummies
                    # only fill a wait if emitted BEFORE the blocked
                    # instruction - (0,1) covers pair2's wait on exp-A(p0)
                    # (~1.7us), (0,3) covers panel-1's wait on exp-B(p0).
                    nwarm = {(0, 1): 4, (0, 3): 2, (1, 1): 1}
                    for _ in range(nwarm.get((p, pi), 0)):
                        nc.tensor.matmul(
                            warm_ps, wz[:, :B], wz, start=True, stop=True
                        )
            final_tiles = emit_pv_final(NPANEL - 1)
            emit_evac_final(NPANEL - 1, final_tiles)

    nc.compile()
    return nc


_NC = None


def _get_nc():
    global _NC
    if _NC is None:
        _NC = _build_nc()
    return _NC


def _make_in_maps(q, k, v):
    q = np.ascontiguousarray(q, dtype=np.float32)
    k = np.ascontiguousarray(k, dtype=np.float32)
    v = np.ascontiguousarray(v, dtype=np.float32)
    in_maps = []
    for h in range(H):
        qT = q[:, h, :].T.astype(NP_BF16)  # [64, 4096]
        qt2 = np.ascontiguousarray(np.concatenate([qT, qT], axis=0))
        kT = k[:, h, :].T.astype(NP_BF16)  # [64, 4096]
        kb = kT.reshape(D, NROW, B)
        kt2 = np.ascontiguousarray(
            np.concatenate([kb[:, 0::2, :], kb[:, 1::2, :]], axis=0)
        )  # [128, 16, 128]: rows 0-63 even blocks, 64-127 odd blocks
        vb = v[:, h, :].reshape(NROW, B, D).transpose(1, 0, 2)  # [128, 32, 64]
        vo = np.concatenate(
            [vb, np.ones((B, NROW, 1), np.float32)], axis=2
        ).astype(NP_BF16)  # [128, 32, 65]
        fc = np.ascontiguousarray(
            np.concatenate(
                [kt2.reshape(2 * D, (NROW // 2) * B)[:, : 2 * B], qt2[:, :512]],
                axis=1,
            )
        )  # [128, 768]: kt blocks 0-3 | qt cols 0-511
        in_maps.append(
            {"qt": qt2, "kt": kt2, "vo": np.ascontiguousarray(vo), "fc": fc}
        )
    return in_maps


def run(q, k, v, trace=False, **trace_kwargs):
    """Returns (out [4096, 8, 64] f32, BassKernelResults)."""
    nc = _get_nc()
    in_maps = _make_in_maps(q, k, v)
    res = run_bass_kernel_spmd(
        nc, in_maps, list(range(H)), trace=trace, **trace_kwargs
    )
    out = np.empty((N, H, D), dtype=np.float32)
    for h in range(H):
        ot = res.results[h]["ot"]  # [65, 4096]
        out[:, h, :] = (ot[:D] / ot[D : D + 1]).T
    return out, res


def kernel(q, k, v, pair_bias=None):
    out, _ = run(q, k, v)
    return out
